# revision 5
# baseline (speedup 1.0000x reference)
"""MinGRU cell on 8 Trainium2 NeuronCores (Bass/Tile).

Math (per batch b, hidden h):
    gz = x @ W_z^T ; gh = x @ W_h^T                 (two GEMMs, K=D=1024)
    z  = sigmoid(gz + b_z)
    h_t = (1 - z_t) * h_{t-1} + z_t * (gh_t + b_h)  (affine scan over T)

Distribution: data-parallel over batch B=16 -> 2 batches per core, weights
replicated; no cross-core communication.

v2: the PE runs ONLY GEMMs (+ a one-time 6.8us of bf16 W transposes).
All x / output transposes are done by the DMA crossbar (InstDmaTransposeAnt,
16-bit only), staged through DRAM:

  x  [t,d] f32 --SWDGE cast--> DRAM bf16 --xbar--> SBUF xT [d,t]
  GEMMs with W^T stationary (bf16, fp32 PSUM accumulation), output [h,t]
  ACT: a = sigmoid(-gz - b_z) = 1-z ; z = sigmoid(gz + b_z)
  DVE: bsc = (gh + b_h) * z ; h = tensor_tensor_scan(a, bsc) along t (bf16 out)
  h [h,t] bf16 --SWDGE--> DRAM scratch --xbar--> SBUF [t,h] --SWDGE cast--> out f32

Per-step budgets (27.3us of GEMM on PE): DMA ~21us, ACT ~13us, DVE ~10us,
Pool ~13us -- PE is the only saturated engine.
"""

import sys

sys.path.insert(0, "/opt/trn_rl_repo")

from contextlib import ExitStack

import numpy as np

import concourse.bass as bass
import concourse.mybir as mybir
import concourse.tile as tile
from concourse import bacc
from concourse.bass import ts, ds
from concourse.bass_utils import run_bass_kernel_spmd
from concourse.masks import make_identity

B, T, D, H = 16, 2048, 1024, 1024
NCORES = 8
B_LOC = B // NCORES  # 2
P = 128
TC = 512  # tokens per step
NSTEP = B_LOC * T // TC  # 8
NTC = T // TC  # 4 steps per batch
TSUB = TC // P  # 4
DC = D // P  # 8 contraction chunks
HC = H // P  # 8 hidden chunks
NWARM = 150  # junk matmuls to open the PE clock gate during the DMA prologue

F32 = mybir.dt.float32
BF16 = mybir.dt.bfloat16
AF = mybir.ActivationFunctionType
OP = mybir.AluOpType

_CACHE = {}


class _State:
    pass


def _mingru_tile(tc, out, x, h0, wz, bz, wh, bh):
    nc = tc.nc
    st = _State()

    with ExitStack() as ctx:
        consts = ctx.enter_context(tc.tile_pool(name="consts", bufs=1))

        id_bf = consts.tile([P, P], BF16)
        make_identity(nc, id_bf)

        bz_sb = consts.tile([P, HC], F32)
        nc.sync.dma_start(out=bz_sb, in_=bz.rearrange("(c p) -> p c", p=P))
        bh_sb = consts.tile([P, HC], F32)
        nc.sync.dma_start(out=bh_sb, in_=bh.rearrange("(c p) -> p c", p=P))
        nbz_sb = consts.tile([P, HC], F32)
        nc.vector.tensor_scalar_mul(nbz_sb, bz_sb, -1.0)
        hp_sb = consts.tile([P, B_LOC * HC], F32)
        nc.sync.dma_start(out=hp_sb, in_=h0.rearrange("b (c p) -> p (b c)", p=P))

        # --- pools -------------------------------------------------------
        # DRAM scratch
        xc_p = ctx.enter_context(tc.tile_pool(name="xc", bufs=3, space="DRAM"))
        scr_p = ctx.enter_context(tc.tile_pool(name="scr", bufs=2, space="DRAM"))
        # SBUF
        wn_p = ctx.enter_context(tc.tile_pool(name="wn", bufs=1))
        xt_p = ctx.enter_context(tc.tile_pool(name="xt", bufs=2))
        azb_p = ctx.enter_context(tc.tile_pool(name="azb", bufs=2))
        scan_p = ctx.enter_context(tc.tile_pool(name="scan", bufs=2))
        on_p = ctx.enter_context(tc.tile_pool(name="on", bufs=3))
        # PSUM pools (pz/ph/pw = 3+3+2 = 8 banks) are opened AFTER the
        # warmup pool below has been closed; set later.
        pz_p = ph_p = pw_p = None

        st.xc = {}  # step -> DRAM bf16 [TC, D]
        st.xt = {}  # step -> [8 xT tiles [P, TC]]
        st.scan = {}  # step -> [8 scan tiles [P, TC] bf16]
        st.scr = {}  # step -> [2 DRAM scratch [4P, TC] bf16]
        st.on = {}  # step -> [4 out-natural tiles [P, H] bf16]
        st.wt = {"z": [None] * HC, "h": [None] * HC}

        def step_bt(s):
            return s // NTC, s % NTC

        def sect_Acast(s):  # x: fp32 DRAM -> bf16 DRAM (SWDGE cast)
            b, tci = step_bt(s)
            xc = xc_p.tile([TC, D], BF16, tag="xc", name=f"xc_{s}")
            nc.gpsimd.dma_start(out=xc, in_=x[b, ds(tci * TC, TC), :])
            st.xc[s] = xc

        def sect_Axbar(s):  # x: DRAM bf16 [t,d] -> SBUF xT [d,t] via xbar
            xc = st.xc.pop(s)
            tiles = []
            for dc in range(DC):
                xt_sb = xt_p.tile([P, TC], BF16, tag=f"xt{dc}", name=f"xt_{s}_{dc}")
                nc.sync.dma_start(
                    out=xt_sb, in_=xc[:, ts(dc, P)], transpose=True
                )
                tiles.append(xt_sb)
            st.xt[s] = tiles

        def load_w(w_ap, which):
            # natural bf16 chunks [128h, D] via SWDGE cast-load
            st_wn = []
            for hc in range(HC):
                t_ = wn_p.tile([P, D], BF16, tag=f"wn_{which}_{hc}",
                               name=f"wn_{which}_{hc}")
                nc.gpsimd.dma_start(out=t_, in_=w_ap[ts(hc, P), :])
                st_wn.append(t_)
            return st_wn

        def build_wt(wn_tiles, which, copy_eng):
            # PE-transpose each [128,128] block (bf16, 1 cyc/row), then copy
            # psum -> SBUF on copy_eng. wt[which][hc][:, dc*P:(dc+1)*P] is
            # the [d, h] stationary block for (hc, dc).
            for hc in range(HC):
                pw = pw_p.tile([P, D], BF16, tag="pw", name=f"pw_{which}_{hc}")
                for dc in range(DC):
                    nc.tensor.transpose(
                        pw[:, ts(dc, P)], wn_tiles[hc][:, ts(dc, P)], id_bf
                    )
                wt_sb = consts.tile([P, D], BF16, name=f"wt_{which}_{hc}")
                copy_eng(wt_sb, pw)
                st.wt[which][hc] = wt_sb

        def gemm(s, hc, which):
            xts = st.xt[s]
            wt = st.wt[which][hc]
            pool = pz_p if which == "z" else ph_p
            psum = pool.tile(
                [P, TC], F32, tag="pz" if which == "z" else "ph",
                name=f"ps{which}_{s}_{hc}",
            )
            for dc in range(DC):
                nc.tensor.matmul(
                    psum,
                    wt[:, ts(dc, P)],
                    xts[dc],
                    start=(dc == 0),
                    stop=(dc == DC - 1),
                )
            return psum

        def sect_post(s, hc, psum_z, psum_h):
            b, tci = step_bt(s)
            a_sb = azb_p.tile([P, TC], F32, tag="a", name=f"a_{s}_{hc}")
            nc.scalar.activation(
                a_sb, psum_z, AF.Sigmoid, bias=nbz_sb[:, hc : hc + 1], scale=-1.0
            )
            z_sb = azb_p.tile([P, TC], F32, tag="z", name=f"z_{s}_{hc}")
            nc.scalar.activation(
                z_sb, psum_z, AF.Sigmoid, bias=bz_sb[:, hc : hc + 1], scale=1.0
            )
            bsc = azb_p.tile([P, TC], F32, tag="b", name=f"b_{s}_{hc}")
            nc.vector.scalar_tensor_tensor(
                bsc, psum_h, bh_sb[:, hc : hc + 1], z_sb, op0=OP.add, op1=OP.mult
            )
            # bf16 scan output: the scan's accumulator state is fp32 in HW
            # regardless of out dtype, so only stored values round (~2^-9).
            sc = scan_p.tile([P, TC], BF16, tag=f"sc{hc}", name=f"sc_{s}_{hc}")
            if tci == 0:
                init = hp_sb[:, b * HC + hc : b * HC + hc + 1]
            else:
                init = st.scan[s - 1][hc][:, TC - 1 : TC]
            nc.vector.tensor_tensor_scan(sc, a_sb, bsc, init, op0=OP.mult, op1=OP.add)
            st.scan.setdefault(s, [None] * HC)[hc] = sc
            # stage to DRAM scratch for the out-xbar (SWDGE, bf16)
            hh = hc // 4
            scrs = st.scr.setdefault(s, [None, None])
            if scrs[hh] is None:
                scrs[hh] = scr_p.tile(
                    [4 * P, TC], BF16, tag=f"scr{hh}", name=f"scr_{s}_{hh}"
                )
            nc.gpsimd.dma_start(out=scrs[hh][ts(hc % 4, P), :], in_=sc)

        def sect_CD(s, z_first=False):
            if z_first:
                # step 0: W_h^T is still being built; run all z GEMMs first
                pzs = [gemm(s, hc, "z") for hc in range(HC)]
                phs = [gemm(s, hc, "h") for hc in range(HC)]
                for hc in range(HC):
                    sect_post(s, hc, pzs[hc], phs[hc])
            else:
                for hc in range(HC):
                    psum_z = gemm(s, hc, "z")
                    psum_h = gemm(s, hc, "h")
                    sect_post(s, hc, psum_z, psum_h)

        def sect_E(s):  # out: scratch [h,t] -> xbar -> [t,h] -> cast-store
            b, tci = step_bt(s)
            scrs = st.scr.pop(s)
            for j in range(TSUB):
                on = on_p.tile([P, H], BF16, tag="on", name=f"on_{s}_{j}")
                for hh in range(2):
                    nc.scalar.dma_start(
                        out=on[:, ds(hh * 4 * P, 4 * P)],
                        in_=scrs[hh][:, ts(j, P)],
                        transpose=True,
                    )
                nc.gpsimd.dma_start(
                    out=out[b, ds(tci * TC + j * P, P), :], in_=on
                )
            if s - 1 in st.scan:
                del st.scan[s - 1]

        # --- prologue ----------------------------------------------------
        # Pool (SWDGE) queue order minimizes time-to-first-GEMM: W_z chunk 0,
        # then the x(0) cast, then the rest of W_z, x(1), W_h.
        wn_z = [wn_p.tile([P, D], BF16, tag="wn_z_0", name="wn_z_0")]
        nc.gpsimd.dma_start(out=wn_z[0], in_=wz[ts(0, P), :])
        sect_Acast(0)
        for hc in range(1, HC):
            t_ = wn_p.tile([P, D], BF16, tag=f"wn_z_{hc}", name=f"wn_z_{hc}")
            nc.gpsimd.dma_start(out=t_, in_=wz[ts(hc, P), :])
            wn_z.append(t_)
        sect_Acast(1)
        wn_h = load_w(wh, "h")

        sect_Axbar(0)

        # HAM warmup: back-to-back junk matmuls so the PE clock gate opens
        # while the prologue DMAs stream. DMA to DRAM keeps it from DCE.
        # The warm PSUM pool must close before pz/ph/pw open (8-bank budget).
        with tc.tile_pool(name="warm", bufs=1, space="PSUM") as warm_p, \
             tc.tile_pool(name="wdram", bufs=1, space="DRAM") as wdram_p:
            junk_ps = warm_p.tile([P, P], F32, name="junk_ps")
            for i in range(NWARM):
                nc.tensor.matmul(
                    junk_ps, id_bf, id_bf, start=(i == 0), stop=(i == NWARM - 1)
                )
            junk_sb = consts.tile([P, P], F32, name="junk_sb")
            nc.vector.tensor_copy(junk_sb, junk_ps)
            junk_dr = wdram_p.tile([P, P], F32, name="junk_dr")
            nc.sync.dma_start(out=junk_dr, in_=junk_sb)

        # PSUM: pz(3) + ph(3) + pw(2) = 8 banks
        pz_p = ctx.enter_context(tc.tile_pool(name="pz", bufs=3, space="PSUM"))
        ph_p = ctx.enter_context(tc.tile_pool(name="ph", bufs=3, space="PSUM"))
        pw_p = ctx.enter_context(tc.tile_pool(name="pw", bufs=2, space="PSUM"))

        # W^T build. W_z copies go on ACT (its queue is otherwise empty until
        # the first sigmoids), W_h copies on DVE (empty until the first bsc).
        # Ordering is deadlock-critical: ACT must not wait on W_h transposes
        # (they sit behind the z GEMMs in the PE queue).
        build_wt(wn_z, "z", nc.scalar.copy)

        # --- steady state ------------------------------------------------
        for s in range(NSTEP):
            if s + 2 < NSTEP:
                sect_Acast(s + 2)
            if s + 1 < NSTEP:
                sect_Axbar(s + 1)
            if s >= 1:
                sect_E(s - 1)
            if s == 0:
                # emit z GEMMs, then W_h build, then h GEMMs
                pzs = [gemm(0, hc, "z") for hc in range(HC)]
                build_wt(wn_h, "h", nc.vector.tensor_copy)
                phs = [gemm(0, hc, "h") for hc in range(HC)]
                for hc in range(HC):
                    sect_post(0, hc, pzs[hc], phs[hc])
            else:
                sect_CD(s)
        sect_E(NSTEP - 1)


def build():
    if "nc" in _CACHE:
        return _CACHE["nc"]
    nc = bacc.Bacc(
        "TRN2", target_bir_lowering=False, debug=False, num_devices=NCORES
    )
    x = nc.dram_tensor("x", [B_LOC, T, D], F32, kind="ExternalInput").ap()
    h0 = nc.dram_tensor("h0", [B_LOC, H], F32, kind="ExternalInput").ap()
    wz = nc.dram_tensor("wz", [H, D], F32, kind="ExternalInput").ap()
    bz = nc.dram_tensor("bz", [H], F32, kind="ExternalInput").ap()
    wh = nc.dram_tensor("wh", [H, D], F32, kind="ExternalInput").ap()
    bh = nc.dram_tensor("bh", [H], F32, kind="ExternalInput").ap()
    out = nc.dram_tensor("out", [B_LOC, T, H], F32, kind="ExternalOutput").ap()
    with tile.TileContext(nc) as tctx:
        _mingru_tile(tctx, out, x, h0, wz, bz, wh, bh)
    nc.compile()
    _CACHE["nc"] = nc
    return nc


def make_in_maps(x, h_prev, W_z, b_z, W_h, b_h):
    x = np.ascontiguousarray(np.asarray(x, dtype=np.float32))
    h_prev = np.ascontiguousarray(np.asarray(h_prev, dtype=np.float32))
    W_z = np.ascontiguousarray(np.asarray(W_z, dtype=np.float32))
    b_z = np.ascontiguousarray(np.asarray(b_z, dtype=np.float32))
    W_h = np.ascontiguousarray(np.asarray(W_h, dtype=np.float32))
    b_h = np.ascontiguousarray(np.asarray(b_h, dtype=np.float32))
    in_maps = []
    for c in range(NCORES):
        sl = slice(c * B_LOC, (c + 1) * B_LOC)
        in_maps.append(
            {
                "x": x[sl],
                "h0": h_prev[sl],
                "wz": W_z,
                "bz": b_z,
                "wh": W_h,
                "bh": b_h,
            }
        )
    return in_maps


def kernel(x, h_prev, W_z, b_z, W_h, b_h, trace=False):
    nc = build()
    in_maps = make_in_maps(x, h_prev, W_z, b_z, W_h, b_h)
    res = run_bass_kernel_spmd(
        nc, in_maps, core_ids=list(range(NCORES)), trace=trace
    )
    out = np.concatenate([r["out"] for r in res.results], axis=0)
    if trace:
        _CACHE["last_results"] = res
    return out


# revision 6
# speedup vs baseline: 1.7756x; 1.7756x over previous
"""MinGRU cell on 8 Trainium2 NeuronCores (Bass/Tile).

Math (per batch b, hidden h):
    gz = x @ W_z^T ; gh = x @ W_h^T                 (two GEMMs, K=D=1024)
    z  = sigmoid(gz + b_z)
    h_t = (1 - z_t) * h_{t-1} + z_t * (gh_t + b_h)  (affine scan over T)

Distribution: data-parallel over batch B=16 -> 2 batches per core, weights
replicated; no cross-core communication.

Per-core pipeline (software-pipelined over 8 steps of 512 tokens):
  x [t,d] --SWDGE cast-load--> bf16 --PE transpose--> xT [d,t]
  GEMMs with W^T stationary (bf16, fp32 PSUM accumulation), output [h, t]
  ACT: a = sigmoid(-gz - b_z) = 1-z ; z = sigmoid(gz + b_z)
  DVE: bsc = (gh + b_h) * z ; h = tensor_tensor_scan(a, bsc) along t
  PE transpose h back to [t, h], copy to fp32, DMA out per h-half.

v3 changes vs the 334us baseline:
  - W^T built in bf16 (1 cyc/row PE transposes instead of 2), streamed
    per-128-row chunk and interleaved with the step-0 GEMM pairs, so the
    first GEMM starts as soon as W chunk 0 + x step 0 have landed (~10us)
    instead of after a bulk W stage (~24us with a 12us PE hole).
  - All PE-transpose scratch unified into one [128,1024] PSUM ring (x-T,
    out-T, W-T): 8 transposes + 1 copy per tile. Frees 2 PSUM banks ->
    pz/ph get 3 banks each for more GEMM slack.
  - Output copies/stores split per (j, h-half) across ACT/DVE and issued
    as 8 smaller DMAs per step for a shorter drain.
  - Warmup junk-matmul chain covers the whole DMA prologue so the PE
    clock (HAM) ramps once and stays at 2.4 GHz.
"""

import sys

sys.path.insert(0, "/opt/trn_rl_repo")

from contextlib import ExitStack

import numpy as np

import concourse.bass as bass
import concourse.mybir as mybir
import concourse.tile as tile
from concourse import bacc
from concourse.bass import ts, ds
from concourse.bass_utils import run_bass_kernel_spmd
from concourse.masks import make_identity

B, T, D, H = 16, 2048, 1024, 1024
NCORES = 8
B_LOC = B // NCORES  # 2
P = 128
TC = 512  # tokens per step
NSTEP = B_LOC * T // TC  # 8
NTC = T // TC  # 4 steps per batch
TSUB = TC // P  # 4
DC = D // P  # 8 contraction chunks
HC = H // P  # 8 hidden chunks
HH = H // 2
NWARM = 80  # junk matmuls to open the PE clock gate during the DMA prologue

F32 = mybir.dt.float32
BF16 = mybir.dt.bfloat16
AF = mybir.ActivationFunctionType
OP = mybir.AluOpType

_CACHE = {}


class _State:
    pass


def _mingru_tile(tc, out, x, h0, wz, bz, wh, bh):
    nc = tc.nc
    st = _State()

    with ExitStack() as ctx:
        consts = ctx.enter_context(tc.tile_pool(name="consts", bufs=1))

        id_bf = consts.tile([P, P], BF16)
        make_identity(nc, id_bf)

        bz_sb = consts.tile([P, HC], F32)
        nc.sync.dma_start(out=bz_sb, in_=bz.rearrange("(c p) -> p c", p=P))
        bh_sb = consts.tile([P, HC], F32)
        nc.sync.dma_start(out=bh_sb, in_=bh.rearrange("(c p) -> p c", p=P))
        nbz_sb = consts.tile([P, HC], F32)
        nc.vector.tensor_scalar_mul(nbz_sb, bz_sb, -1.0)
        hp_sb = consts.tile([P, B_LOC * HC], F32)
        nc.sync.dma_start(out=hp_sb, in_=h0.rearrange("b (c p) -> p (b c)", p=P))

        # --- SBUF pools --------------------------------------------------
        wn_p = ctx.enter_context(tc.tile_pool(name="wn", bufs=1))
        xn_p = ctx.enter_context(tc.tile_pool(name="xn", bufs=2))
        xt_p = ctx.enter_context(tc.tile_pool(name="xt", bufs=2))
        azb_p = ctx.enter_context(tc.tile_pool(name="azb", bufs=2))
        scan_p = ctx.enter_context(tc.tile_pool(name="scan", bufs=2))
        on_p = ctx.enter_context(tc.tile_pool(name="on", bufs=2))

        st.xn = {}  # step -> [4 natural x tiles [P, D] bf16]
        st.xt = {}  # step -> [4 xT pair tiles [P, 2*TC] bf16] (dc pair p)
        st.scan = {}  # step -> [8 scan tiles [P, TC] bf16]
        st.wt = {"z": [None] * HC, "h": [None] * HC}
        st.wn = {"z": [None] * HC, "h": [None] * HC}

        def step_bt(s):
            return s // NTC, s % NTC

        def sect_A(s):  # x cast-loads (SWDGE)
            b, tci = step_bt(s)
            tiles = []
            for j in range(TSUB):
                xt_nat = xn_p.tile([P, D], BF16, tag=f"xn{j}", name=f"xn_{s}_{j}")
                nc.gpsimd.dma_start(
                    out=xt_nat, in_=x[b, ds(tci * TC + j * P, P), :]
                )
                tiles.append(xt_nat)
            st.xn[s] = tiles

        def load_w_chunk(hc):  # SWDGE cast-load one [128h, D] chunk of each W
            for w_ap, wi in ((wz, "z"), (wh, "h")):
                t_ = wn_p.tile([P, D], BF16, tag=f"wn_{wi}_{hc}",
                               name=f"wn_{wi}_{hc}")
                nc.gpsimd.dma_start(out=t_, in_=w_ap[ts(hc, P), :])
                st.wn[wi][hc] = t_

        def build_wt(wi, hc):  # PE-transpose one W chunk (bf16), copy to SBUF
            pw = pxt_p.tile([P, D], BF16, tag="pxt", name=f"pw_{wi}_{hc}")
            wnt = st.wn[wi][hc]
            for dc in range(DC):
                nc.tensor.transpose(pw[:, ts(dc, P)], wnt[:, ts(dc, P)], id_bf)
            wt_sb = consts.tile([P, D], BF16, name=f"wt_{wi}_{hc}")
            # W_z copies on ACT, W_h copies on DVE (keeps both queues clear
            # of cross-waits against the step-0 GEMM stream).
            if wi == "z":
                nc.scalar.copy(wt_sb, pw)
            else:
                nc.vector.tensor_copy(wt_sb, pw)
            st.wt[wi][hc] = wt_sb

        def sect_B(s):  # x transposes (PE) into [P, 2*TC] pair tiles
            xn = st.xn.pop(s)
            tiles = []
            for p_ in range(DC // 2):
                pxt = pxt_p.tile([P, 2 * TC], BF16, tag="pxt",
                                 name=f"pxt_{s}_{p_}")
                for q in range(2):
                    for j in range(TSUB):
                        nc.tensor.transpose(
                            pxt[:, ds(q * TC + j * P, P)],
                            xn[j][:, ts(2 * p_ + q, P)],
                            id_bf,
                        )
                xt_sb = xt_p.tile([P, 2 * TC], BF16, tag=f"xt{p_}",
                                  name=f"xt_{s}_{p_}")
                nc.scalar.copy(xt_sb, pxt)
                tiles.append(xt_sb)
            st.xt[s] = tiles

        def gemm(s, hc, which):
            xts = st.xt[s]
            wt = st.wt[which][hc]
            pool = pz_p if which == "z" else ph_p
            psum = pool.tile(
                [P, TC], F32, tag="pz" if which == "z" else "ph",
                name=f"ps{which}_{s}_{hc}",
            )
            for dc in range(DC):
                nc.tensor.matmul(
                    psum,
                    wt[:, ts(dc, P)],
                    xts[dc // 2][:, ds((dc % 2) * TC, TC)],
                    start=(dc == 0),
                    stop=(dc == DC - 1),
                )
            return psum

        def sect_post(s, hc, psum_z, psum_h):
            b, tci = step_bt(s)
            a_sb = azb_p.tile([P, TC], F32, tag="a", name=f"a_{s}_{hc}")
            nc.scalar.activation(
                a_sb, psum_z, AF.Sigmoid, bias=nbz_sb[:, hc : hc + 1], scale=-1.0
            )
            z_sb = azb_p.tile([P, TC], F32, tag="z", name=f"z_{s}_{hc}")
            nc.scalar.activation(
                z_sb, psum_z, AF.Sigmoid, bias=bz_sb[:, hc : hc + 1], scale=1.0
            )
            bsc = azb_p.tile([P, TC], F32, tag="b", name=f"b_{s}_{hc}")
            nc.vector.scalar_tensor_tensor(
                bsc, psum_h, bh_sb[:, hc : hc + 1], z_sb, op0=OP.add, op1=OP.mult
            )
            # bf16 scan output: the scan's accumulator state is fp32 in HW
            # regardless of out dtype, so only stored values round (~2^-9).
            sc = scan_p.tile([P, TC], BF16, tag=f"sc{hc}", name=f"sc_{s}_{hc}")
            if tci == 0:
                init = hp_sb[:, b * HC + hc : b * HC + hc + 1]
            else:
                init = st.scan[s - 1][hc][:, TC - 1 : TC]
            nc.vector.tensor_tensor_scan(sc, a_sb, bsc, init, op0=OP.mult, op1=OP.add)
            st.scan.setdefault(s, [None] * HC)[hc] = sc

        def sect_E(s):  # out transposes (PE), copy to f32, store per h-half
            b, tci = step_bt(s)
            scans = st.scan[s]
            for j in range(TSUB):
                po = pxt_p.tile([P, H], BF16, tag="pxt", name=f"po_{s}_{j}")
                for hc in range(HC):
                    nc.tensor.transpose(
                        po[:, ts(hc, P)], scans[hc][:, ts(j, P)], id_bf
                    )
                for hh in range(2):
                    on = on_p.tile([P, HH], F32, tag=f"on{hh}",
                                   name=f"on_{s}_{j}_{hh}")
                    # split the psum->fp32 casts across ACT / DVE
                    if hh == 0:
                        nc.scalar.copy(on, po[:, ds(0, HH)])
                    else:
                        nc.vector.tensor_copy(on, po[:, ds(HH, HH)])
                    nc.sync.dma_start(
                        out=out[b, ds(tci * TC + j * P, P), ds(hh * HH, HH)],
                        in_=on,
                    )
            if s - 1 in st.scan:
                del st.scan[s - 1]

        # --- prologue ----------------------------------------------------
        # Pool (SWDGE) queue order = consumption order: W pair 0, x step 0,
        # then W pairs 1..7 (x step 1 is emitted early in the step-0 body).
        load_w_chunk(0)
        sect_A(0)
        for hc in range(1, HC):
            load_w_chunk(hc)

        # HAM warmup: back-to-back junk matmuls so the PE clock gate opens
        # while the prologue DMAs stream. The warm PSUM pool must close
        # before pz/ph/pxt open (8-bank budget).
        with tc.tile_pool(name="warm", bufs=1, space="PSUM") as warm_p, \
             tc.tile_pool(name="wdram", bufs=1, space="DRAM") as wdram_p:
            junk_ps = warm_p.tile([P, P], F32, name="junk_ps")
            for i in range(NWARM):
                nc.tensor.matmul(
                    junk_ps, id_bf, id_bf, start=(i == 0), stop=(i == NWARM - 1)
                )
            junk_sb = consts.tile([P, P], F32, name="junk_sb")
            nc.vector.tensor_copy(junk_sb, junk_ps)
            junk_dr = wdram_p.tile([P, P], F32, name="junk_dr")
            nc.sync.dma_start(out=junk_dr, in_=junk_sb)

        # PSUM: pz(3) + ph(3) + pxt(2) = 8 banks
        pz_p = ctx.enter_context(tc.tile_pool(name="pz", bufs=3, space="PSUM"))
        ph_p = ctx.enter_context(tc.tile_pool(name="ph", bufs=3, space="PSUM"))
        pxt_p = ctx.enter_context(tc.tile_pool(name="pxt", bufs=2, space="PSUM"))

        # W^T chunks 0-1 ahead of the GEMM stream (2-chunk lookahead).
        for hc in range(2):
            build_wt("z", hc)
            build_wt("h", hc)
        sect_B(0)

        # --- steady state ------------------------------------------------
        for s in range(NSTEP):
            if s + 1 < NSTEP:
                sect_A(s + 1)
            for hc in range(HC):
                if s == 0 and hc + 2 < HC:
                    build_wt("z", hc + 2)
                    build_wt("h", hc + 2)
                psum_z = gemm(s, hc, "z")
                psum_h = gemm(s, hc, "h")
                sect_post(s, hc, psum_z, psum_h)
                if s >= 1 and hc == 2:
                    sect_E(s - 1)
                if s + 1 < NSTEP and hc == 4:
                    sect_B(s + 1)
        sect_E(NSTEP - 1)


def build():
    if "nc" in _CACHE:
        return _CACHE["nc"]
    nc = bacc.Bacc(
        "TRN2", target_bir_lowering=False, debug=False, num_devices=NCORES
    )
    x = nc.dram_tensor("x", [B_LOC, T, D], F32, kind="ExternalInput").ap()
    h0 = nc.dram_tensor("h0", [B_LOC, H], F32, kind="ExternalInput").ap()
    wz = nc.dram_tensor("wz", [H, D], F32, kind="ExternalInput").ap()
    bz = nc.dram_tensor("bz", [H], F32, kind="ExternalInput").ap()
    wh = nc.dram_tensor("wh", [H, D], F32, kind="ExternalInput").ap()
    bh = nc.dram_tensor("bh", [H], F32, kind="ExternalInput").ap()
    out = nc.dram_tensor("out", [B_LOC, T, H], F32, kind="ExternalOutput").ap()
    with tile.TileContext(nc) as tctx:
        _mingru_tile(tctx, out, x, h0, wz, bz, wh, bh)
    nc.compile()
    _CACHE["nc"] = nc
    return nc


def make_in_maps(x, h_prev, W_z, b_z, W_h, b_h):
    x = np.ascontiguousarray(np.asarray(x, dtype=np.float32))
    h_prev = np.ascontiguousarray(np.asarray(h_prev, dtype=np.float32))
    W_z = np.ascontiguousarray(np.asarray(W_z, dtype=np.float32))
    b_z = np.ascontiguousarray(np.asarray(b_z, dtype=np.float32))
    W_h = np.ascontiguousarray(np.asarray(W_h, dtype=np.float32))
    b_h = np.ascontiguousarray(np.asarray(b_h, dtype=np.float32))
    in_maps = []
    for c in range(NCORES):
        sl = slice(c * B_LOC, (c + 1) * B_LOC)
        in_maps.append(
            {
                "x": x[sl],
                "h0": h_prev[sl],
                "wz": W_z,
                "bz": b_z,
                "wh": W_h,
                "bh": b_h,
            }
        )
    return in_maps


def kernel(x, h_prev, W_z, b_z, W_h, b_h, trace=False):
    nc = build()
    in_maps = make_in_maps(x, h_prev, W_z, b_z, W_h, b_h)
    res = run_bass_kernel_spmd(
        nc, in_maps, core_ids=list(range(NCORES)), trace=trace
    )
    out = np.concatenate([r["out"] for r in res.results], axis=0)
    if trace:
        _CACHE["last_results"] = res
    return out
